# revision 1
# baseline (speedup 1.0000x reference)
"""Trainium2 Bass kernel for CrossModalAttention.

Reference computation (per sample n, data-parallel over 8 cores):
  img_mean[c,t]  = mean_v image[c,t,v]
  Q[r,t]         = w_iq @ img_mean + b_iq
  K[r,t]         = w_mq @ (block-mean of motion) + b_mq
  Vv[c,t,v]      = w_iv @ image + b_iv
  s[r,i,j]       = tanh(K[r,i] - Q[r,j])
  att[c,i,j]     = (w_att @ s + b_att)/T1 + I
  y[c,i,v]       = sum_j att[c,i,j] * Vv[c,j,v]

Kernel strategy per core (one sample):
 - stream image per (t, half): PE matmul Vv (fp32r full-rate), gpsimd reduce
   for img_sum, ACT copy PSUM->SBUF with fused b_iv, then SBUF->SBUF "fold"
   DMA into V_fold layout [p=16*cl+j, g*784+v] with channel c = 32*cl+g.
 - tiny attention chain -> att matrix per channel, scattered via DMA into a
   block-diagonal lhsT buffer (32 groups of 8 channels).
 - 32 block-diagonal matmuls y_g = att_bd_g.T @ V_fold_g (full 128-K PE use),
   DVE copy PSUM->SBUF, DMA out with scatter to the natural y layout.
"""

import numpy as np

N, C, T1, H, W = 8, 256, 16, 28, 28
HW = H * W
T2, V = 64, 25
REL = 32
CH = 128  # partition half of C
NCORES = 8
NG = 32  # channel groups (c = 32*cl + g)

_CACHE = {}


def _build():
    from contextlib import ExitStack
    from concourse import bass, mybir, tile, bacc, masks

    dt = mybir.dt
    f32 = dt.float32
    f32r = dt.float32r
    AF = mybir.ActivationFunctionType

    nc = bacc.Bacc("TRN2", target_bir_lowering=False, debug=False)

    image = nc.dram_tensor("image", [C, T1, HW], f32, kind="ExternalInput").ap()
    motion = nc.dram_tensor("motion", [C, T2 * V], f32, kind="ExternalInput").ap()
    w_iv = nc.dram_tensor("w_iv", [C, C], f32, kind="ExternalInput").ap()
    b_iv = nc.dram_tensor("b_iv", [C], f32, kind="ExternalInput").ap()
    w_iq = nc.dram_tensor("w_iq", [REL, C], f32, kind="ExternalInput").ap()
    b_iq = nc.dram_tensor("b_iq", [REL], f32, kind="ExternalInput").ap()
    w_mq = nc.dram_tensor("w_mq", [REL, C], f32, kind="ExternalInput").ap()
    b_mq = nc.dram_tensor("b_mq", [REL], f32, kind="ExternalInput").ap()
    w_att = nc.dram_tensor("w_att", [C, REL], f32, kind="ExternalInput").ap()
    b_att = nc.dram_tensor("b_att", [C], f32, kind="ExternalInput").ap()
    y = nc.dram_tensor("y", [C, T1, HW], f32, kind="ExternalOutput").ap()

    with tile.TileContext(nc) as tc, ExitStack() as ctx:
        const = ctx.enter_context(tc.tile_pool(name="const", bufs=1))
        img_pool = ctx.enter_context(tc.tile_pool(name="imgp", bufs=4))
        stg_pool = ctx.enter_context(tc.tile_pool(name="stgp", bufs=4))
        y_pool = ctx.enter_context(tc.tile_pool(name="yp", bufs=3))
        ps_pool = ctx.enter_context(tc.tile_pool(name="ps", bufs=3, space="PSUM"))
        ps_small = ctx.enter_context(tc.tile_pool(name="pss", bufs=2, space="PSUM"))

        # ---------------- setup: constants ----------------
        ident = const.tile([128, 128], f32, name="ident", tag="ident")
        masks.make_identity(nc, ident[:])

        eye = const.tile([128, 256], f32r, name="eye", tag="eye")
        nc.vector.memset(eye[:].bitcast(f32), 0.0)
        for j in range(T1):
            nc.vector.memset(eye[:, j * 17 : j * 17 + 1].bitcast(f32), 1.0)

        # biases
        b_iq_sb = const.tile([REL, 1], f32, name="b_iq_sb", tag="b_iq_sb")
        nc.sync.dma_start(b_iq_sb[:], b_iq[:])
        b_mq_sb = const.tile([REL, 1], f32, name="b_mq_sb", tag="b_mq_sb")
        nc.sync.dma_start(b_mq_sb[:], b_mq[:])
        b_iv_sb = const.tile([128, 2], f32, name="b_iv_sb", tag="b_iv_sb")
        b_att_sb = const.tile([128, 2], f32, name="b_att_sb", tag="b_att_sb")
        for h in range(2):
            nc.sync.dma_start(b_iv_sb[:, h : h + 1], b_iv[h * CH : (h + 1) * CH])
            nc.sync.dma_start(b_att_sb[:, h : h + 1], b_att[h * CH : (h + 1) * CH])
        nc.scalar.mul(b_att_sb[:], b_att_sb[:], 1.0 / T1)

        # ---------------- setup: weight transposes ----------------
        # w_iv -> lhsT_iv blocks [c'(128), d(128)] at cols (kh*2+h2)*128
        w_iv_sb = const.tile([128, 512], f32, name="w_iv_sb", tag="w_iv_sb")
        for h in range(2):
            nc.sync.dma_start(
                w_iv_sb[:, h * 256 : (h + 1) * 256], w_iv[h * CH : (h + 1) * CH, :]
            )
        lhsT_iv = const.tile([128, 512], f32r, name="lhsT_iv", tag="lhsT_iv")
        for kh in range(2):
            for h2 in range(2):
                tp = ps_small.tile([128, 256], f32, name="tp", tag="pss")
                nc.tensor.transpose(
                    tp[:, 0:128],
                    w_iv_sb[:, h2 * 256 + kh * 128 : h2 * 256 + (kh + 1) * 128],
                    ident[:],
                )
                nc.scalar.copy(
                    lhsT_iv[:, (kh * 2 + h2) * 128 : (kh * 2 + h2 + 1) * 128],
                    tp[:, 0:128],
                )

        # w_iq / w_mq -> lhsT [c'(128), r(32)] at cols kh*32, scaled by 1/HW, 1/100
        w_iq_sb = const.tile([REL, C], f32, name="w_iq_sb", tag="w_iq_sb")
        nc.sync.dma_start(w_iq_sb[:], w_iq[:])
        w_mq_sb = const.tile([REL, C], f32, name="w_mq_sb", tag="w_mq_sb")
        nc.sync.dma_start(w_mq_sb[:], w_mq[:])
        lhsT_iq = const.tile([128, 64], f32r, name="lhsT_iq", tag="lhsT_iq")
        lhsT_mq = const.tile([128, 64], f32r, name="lhsT_mq", tag="lhsT_mq")
        for kh in range(2):
            tp = ps_small.tile([128, 256], f32, name="tp", tag="pss")
            nc.tensor.transpose(
                tp[:, 0:32], w_iq_sb[:, kh * 128 : (kh + 1) * 128], ident[0:32, 0:32]
            )
            nc.scalar.mul(lhsT_iq[:, kh * 32 : (kh + 1) * 32], tp[:, 0:32], 1.0 / HW)
            tp2 = ps_small.tile([128, 256], f32, name="tp2", tag="pss")
            nc.tensor.transpose(
                tp2[:, 0:32], w_mq_sb[:, kh * 128 : (kh + 1) * 128], ident[0:32, 0:32]
            )
            nc.scalar.mul(
                lhsT_mq[:, kh * 32 : (kh + 1) * 32],
                tp2[:, 0:32],
                1.0 / ((T2 // T1) * V),
            )

        # w_att [C, REL] -> lhsT_att [r(32), c(256)]
        w_att_sb = const.tile([128, 64], f32, name="w_att_sb", tag="w_att_sb")
        for h in range(2):
            nc.sync.dma_start(
                w_att_sb[:, h * 32 : (h + 1) * 32], w_att[h * CH : (h + 1) * CH, :]
            )
        lhsT_att = const.tile([REL, 256], f32r, name="lhsT_att", tag="lhsT_att")
        for h in range(2):
            tp = ps_small.tile([128, 256], f32, name="tp", tag="pss")
            nc.tensor.transpose(
                tp[0:32, 0:128], w_att_sb[:, h * 32 : (h + 1) * 32], ident[:]
            )
            nc.scalar.copy(lhsT_att[:, h * 128 : (h + 1) * 128], tp[0:32, 0:128])

        # block-diagonal att buffer, zeroed once
        lhsT_bd = const.tile([128, NG * 128], f32r, name="lhsT_bd", tag="lhsT_bd")
        nc.gpsimd.memset(lhsT_bd[:].bitcast(f32), 0.0)

        # ---------------- motion pooling ----------------
        mot_pool = const.tile([128, 32], f32r, name="mot_pool", tag="mot_pool")
        for h in range(2):
            mot = const.tile([128, T2 * V], f32, name="mot", tag=f"mot{h}")
            nc.sync.dma_start(mot[:], motion[h * CH : (h + 1) * CH, :])
            with nc.allow_low_precision(reason="f32r output, fp32 accumulate"):
                nc.vector.reduce_sum(
                    mot_pool[:, h * T1 : (h + 1) * T1],
                    mot[:].rearrange("p (t q) -> p t q", q=(T2 // T1) * V),
                    axis=mybir.AxisListType.X,
                )

        # ---------------- phase 1: stream image ----------------
        V_fold = const.tile([128, NG * HW], f32r, name="V_fold", tag="V_fold")
        img_sum = const.tile([128, 32], f32r, name="img_sum", tag="img_sum")

        for t in range(T1):
            imgs = []
            for h in range(2):
                img = img_pool.tile([128, HW], f32r, name="img", tag="img")
                nc.sync.dma_start(
                    img[:], image[h * CH : (h + 1) * CH, t, :].bitcast(f32r)
                )
                imgs.append(img)
                # per-(c,t) spatial sum for Q
                with nc.allow_low_precision(reason="f32r output, fp32 accumulate"):
                    nc.vector.reduce_sum(
                        img_sum[:, h * T1 + t : h * T1 + t + 1],
                        img[:].bitcast(f32),
                        axis=mybir.AxisListType.X,
                    )
            for h2 in range(2):
                vv = ps_pool.tile([128, 1024], f32, name="vv", tag="mm")
                for kh in range(2):
                    for cs, pcol in ((0, 0), (392, 512)):
                        nc.tensor.matmul(
                            vv[:, pcol : pcol + 392],
                            lhsT_iv[:, (kh * 2 + h2) * 128 : (kh * 2 + h2 + 1) * 128],
                            imgs[kh][:, cs : cs + 392],
                            start=(kh == 0),
                            stop=(kh == 1),
                        )
                stg = stg_pool.tile([128, HW], f32r, name="stg", tag="stg")
                nc.scalar.activation(
                    stg[:].rearrange("p (c x) -> p c x", x=392),
                    vv[:].rearrange("p (c x) -> p c x", x=512)[:, :, 0:392],
                    AF.Identity,
                    bias=b_iv_sb[:, h2 : h2 + 1],
                )
                # fold: V_fold[16*(4*h2+cl)+t, g*HW+v] = stg[32*cl+g, v]
                # one DMA per (t, h2): src [128, 784] -> dst [4 partitions
                # (step 16), 25088 contiguous]
                nc.scalar.dma_start(
                    V_fold[:].rearrange("(cl r) q -> cl r q", r=T1)[
                        4 * h2 : 4 * h2 + 4, t
                    ],
                    stg[:],
                )

        # ---------------- phase 2: attention ----------------
        q_ps = ps_small.tile([128, 256], f32, name="q_ps", tag="pss")
        k_ps = ps_small.tile([128, 256], f32, name="k_ps", tag="pss")
        for kh in range(2):
            nc.tensor.matmul(
                q_ps[0:REL, 0:T1],
                lhsT_iq[:, kh * 32 : (kh + 1) * 32],
                img_sum[:, kh * T1 : (kh + 1) * T1],
                start=(kh == 0),
                stop=(kh == 1),
            )
            nc.tensor.matmul(
                k_ps[0:REL, 0:T1],
                lhsT_mq[:, kh * 32 : (kh + 1) * 32],
                mot_pool[:, kh * T1 : (kh + 1) * T1],
                start=(kh == 0),
                stop=(kh == 1),
            )
        q_sb = const.tile([REL, T1], f32, name="q_sb", tag="q_sb")
        nc.scalar.activation(
            q_sb[:], q_ps[0:REL, 0:T1], AF.Identity, bias=b_iq_sb[:, 0:1]
        )
        k_sb = const.tile([REL, T1], f32, name="k_sb", tag="k_sb")
        nc.scalar.activation(
            k_sb[:], k_ps[0:REL, 0:T1], AF.Identity, bias=b_mq_sb[:, 0:1]
        )

        # s2[r, j*16+i] = tanh(K[r,i] - Q[r,j])
        diff = const.tile([REL, 256], f32, name="diff", tag="diff")
        nc.vector.tensor_sub(
            diff[:].rearrange("p (j i) -> p j i", i=T1),
            k_sb[:].unsqueeze(1).broadcast_to((REL, T1, T1)),
            q_sb[:].unsqueeze(2).broadcast_to((REL, T1, T1)),
        )
        s2 = const.tile([REL, 256], f32r, name="s2", tag="s2")
        nc.scalar.activation(s2[:], diff[:], AF.Tanh)

        # att_sb_h[c_loc, j*16+i] = att[c, i, j] = (w_att@s2 + b_att)/16 + I
        att_sbs = []
        for h in range(2):
            a_ps = ps_small.tile([128, 256], f32, name="a_ps", tag="pss")
            nc.tensor.matmul(
                a_ps[:, 0:256],
                lhsT_att[:, h * 128 : (h + 1) * 128],
                s2[:],
            )
            att_sb = const.tile([128, 256], f32r, name="att_sb", tag=f"att_sb{h}")
            nc.scalar.activation(
                att_sb[:],
                a_ps[:, 0:256],
                AF.Identity,
                scale=1.0 / T1,
                bias=b_att_sb[:, h : h + 1],
            )
            nc.vector.tensor_add(att_sb[:], att_sb[:], eye[:])
            att_sbs.append(att_sb)

        # scatter into block-diagonal lhsT:
        # lhsT_bd[16*cl+j, g*128+16*cl+i] = att_sb[32*cl+g, j*16+i]
        # dst col offset 16*cl is partition-dependent (block diagonal), so the
        # scatter cannot coalesce across cl: per-(cl, j) DMAs, alternating
        # between the two HWDGE rings.
        for h in range(2):
            for j in range(T1):
                for cl_loc in range(4):
                    cl = 4 * h + cl_loc
                    s = att_sbs[h][
                        32 * cl_loc : 32 * cl_loc + 32, j * T1 : (j + 1) * T1
                    ]
                    d = lhsT_bd[16 * cl + j : 16 * cl + j + 1, :].rearrange(
                        "p (g c) -> p g c", c=128
                    )[:, :, 16 * cl : 16 * cl + 16]
                    eng = nc.scalar if (j + cl_loc) % 2 == 0 else nc.sync
                    eng.dma_start(d, s)

        # ---------------- phase 3: y = att_bd.T @ V_fold ----------------
        for g in range(NG):
            yp = ps_pool.tile([128, 1024], f32, name="yp", tag="mm")
            for cs, pcol in ((0, 0), (392, 512)):
                nc.tensor.matmul(
                    yp[:, pcol : pcol + 392],
                    lhsT_bd[:, g * 128 : (g + 1) * 128],
                    V_fold[:, g * HW + cs : g * HW + cs + 392],
                )
            y_sb = y_pool.tile([128, HW], f32, name="y_sb", tag="y_sb")
            copy_eng = nc.scalar if (g % 8) < 5 else nc.vector
            if copy_eng is nc.scalar:
                nc.scalar.copy(
                    y_sb[:].rearrange("p (c x) -> p c x", x=392),
                    yp[:].rearrange("p (c x) -> p c x", x=512)[:, :, 0:392],
                )
            else:
                nc.vector.tensor_copy(
                    y_sb[:].rearrange("p (c x) -> p c x", x=392),
                    yp[:].rearrange("p (c x) -> p c x", x=512)[:, :, 0:392],
                )
            out_eng = nc.sync if g % 2 == 0 else nc.scalar
            out_eng.dma_start(
                y[:].rearrange("(cl g) t v -> cl g t v", g=NG)[:, g],
                y_sb[:],
            )

    nc.compile()
    return nc


def _get_nc():
    if "nc" not in _CACHE:
        _CACHE["nc"] = _build()
    return _CACHE["nc"]


def kernel(**inputs) -> np.ndarray:
    from concourse.bass_utils import run_bass_kernel_spmd

    nc = _get_nc()

    image = np.ascontiguousarray(np.asarray(inputs["image"], dtype=np.float32))
    motion = np.ascontiguousarray(np.asarray(inputs["motion"], dtype=np.float32))
    shared = {
        k: np.ascontiguousarray(np.asarray(inputs[k], dtype=np.float32))
        for k in ("w_iv", "b_iv", "w_iq", "b_iq", "w_mq", "b_mq", "w_att", "b_att")
    }
    in_maps = []
    for n in range(NCORES):
        m = {
            "image": image[n].reshape(C, T1, HW),
            "motion": motion[n].reshape(C, T2 * V),
        }
        m.update(shared)
        in_maps.append(m)

    res = run_bass_kernel_spmd(nc, in_maps, core_ids=list(range(NCORES)))
    out = np.stack([res.results[n]["y"] for n in range(NCORES)], axis=0)
    return out.reshape(N, C, T1, H, W)



# revision 12
# speedup vs baseline: 1.6407x; 1.6407x over previous
"""Trainium2 Bass kernel for CrossModalAttention.

Reference computation (per sample n, data-parallel over 8 cores):
  img_mean[c,t]  = mean_v image[c,t,v]
  Q[r,t]         = w_iq @ img_mean + b_iq
  K[r,t]         = w_mq @ (block-mean of motion) + b_mq
  Vv[c,t,v]      = w_iv @ image + b_iv
  s[r,i,j]       = tanh(K[r,i] - Q[r,j])
  att[c,i,j]     = (w_att @ s + b_att)/T1 + I
  y[c,i,v]       = sum_j att[c,i,j] * Vv[c,j,v]

v2 strategy per core (one sample), latency-optimized:
 - whole image resident in SBUF (100KB/partition), loaded as 16 fat chunks on
   two DMA queues (sync + vector) so the tensor engine streams back-to-back.
 - per (t, h2): 4 matmuls (f32r full-rate) into a PSUM tile; bias+fp16
   convert on ACT (h2=0) / Pool (h2=1); fold-DMA into V_fold fp16
   [p=16*cl+t, g*784+v] (c = 32*cl+g) on the producing engine's queue.
 - img_sum via DVE per-t reduces; motion pooled early on DVE; K precomputed.
 - attention chain with the identity and 1/T1 fused into an extended
   K=33 matmul (extra eye row in s2 / ones row in lhsT_att).
 - att scatter into block-diagonal lhsT_bd (fp16) as 8 batched DMAs
   (one per cl) using a partition-middle source AP.
 - 32 block-diagonal matmuls y_g = att_bd_g.T @ V_fold_g (fp16, one
   ldweights each), PSUM->SBUF copies round-robined over ACT/DVE/Pool,
   output DMAs alternating sync/vector queues.
"""

import numpy as np

N, C, T1, H, W = 8, 256, 16, 28, 28
HW = H * W
T2, V = 64, 25
REL = 32
CH = 128  # partition half of C
NCORES = 8
NG = 32  # channel groups (c = 32*cl + g)

_CACHE = {}


def _build():
    from contextlib import ExitStack
    from concourse import bass, mybir, tile, bacc, masks

    dt = mybir.dt
    f32 = dt.float32
    f32r = dt.float32r
    fp16 = dt.float16
    AF = mybir.ActivationFunctionType

    nc = bacc.Bacc("TRN2", target_bir_lowering=False, debug=False)

    image = nc.dram_tensor("image", [C, T1, HW], f32, kind="ExternalInput").ap()
    motion = nc.dram_tensor("motion", [C, T2 * V], f32, kind="ExternalInput").ap()
    w_iv = nc.dram_tensor("w_iv", [C, C], f32, kind="ExternalInput").ap()
    b_iv = nc.dram_tensor("b_iv", [C], f32, kind="ExternalInput").ap()
    w_iq = nc.dram_tensor("w_iq", [REL, C], f32, kind="ExternalInput").ap()
    b_iq = nc.dram_tensor("b_iq", [REL], f32, kind="ExternalInput").ap()
    w_mq = nc.dram_tensor("w_mq", [REL, C], f32, kind="ExternalInput").ap()
    b_mq = nc.dram_tensor("b_mq", [REL], f32, kind="ExternalInput").ap()
    w_att = nc.dram_tensor("w_att", [C, REL], f32, kind="ExternalInput").ap()
    b_att = nc.dram_tensor("b_att", [C], f32, kind="ExternalInput").ap()
    y = nc.dram_tensor("y", [C, T1, HW], f32, kind="ExternalOutput").ap()

    with tile.TileContext(nc) as tc, ExitStack() as ctx:
        const = ctx.enter_context(tc.tile_pool(name="const", bufs=1))
        stg_pool = ctx.enter_context(tc.tile_pool(name="stgp", bufs=4))
        y_pool = ctx.enter_context(tc.tile_pool(name="yp", bufs=4))
        ps_pool = ctx.enter_context(tc.tile_pool(name="ps", bufs=4, space="PSUM"))

        # ---------------- input DMAs first: weights, then image stream -----
        w_iv_sb = const.tile([128, 512], f32, name="w_iv_sb", tag="w_iv_sb")
        for h in range(2):
            nc.sync.dma_start(
                w_iv_sb[:, h * 256 : (h + 1) * 256], w_iv[h * CH : (h + 1) * CH, :]
            )
        w_iq_sb = const.tile([REL, C], f32, name="w_iq_sb", tag="w_iq_sb")
        nc.sync.dma_start(w_iq_sb[:], w_iq[:])
        w_mq_sb = const.tile([REL, C], f32, name="w_mq_sb", tag="w_mq_sb")
        nc.sync.dma_start(w_mq_sb[:], w_mq[:])
        w_att_sb = const.tile([128, 64], f32, name="w_att_sb", tag="w_att_sb")
        for h in range(2):
            nc.sync.dma_start(
                w_att_sb[:, h * 32 : (h + 1) * 32], w_att[h * CH : (h + 1) * CH, :]
            )
        b_iq_sb = const.tile([REL, 1], f32, name="b_iq_sb", tag="b_iq_sb")
        nc.scalar.dma_start(b_iq_sb[:], b_iq[:])
        b_mq_sb = const.tile([REL, 1], f32, name="b_mq_sb", tag="b_mq_sb")
        nc.scalar.dma_start(b_mq_sb[:], b_mq[:])
        b_iv_sb = const.tile([128, 2], f32, name="b_iv_sb", tag="b_iv_sb")
        b_att_sb = const.tile([128, 2], f32, name="b_att_sb", tag="b_att_sb")
        for h in range(2):
            nc.scalar.dma_start(b_iv_sb[:, h : h + 1], b_iv[h * CH : (h + 1) * CH])
            nc.scalar.dma_start(b_att_sb[:, h : h + 1], b_att[h * CH : (h + 1) * CH])

        # whole image resident: cols h*T1*HW + t*HW + v
        img_sb = const.tile([128, 2 * T1 * HW], f32r, name="img_sb", tag="img_sb")
        for tq in range(8):  # 2-t chunks, both halves interleaved on sync queue
            for h in range(2):
                nc.sync.dma_start(
                    img_sb[
                        :,
                        (h * T1 + 2 * tq) * HW : (h * T1 + 2 * tq + 2) * HW,
                    ],
                    image[h * CH : (h + 1) * CH, 2 * tq : 2 * tq + 2, :].bitcast(
                        f32r
                    ),
                )
        # motion on the Pool queue
        mot = const.tile([128, 2 * T2 * V], f32, name="mot", tag="mot")
        for h in range(2):
            nc.gpsimd.dma_start(
                mot[:, h * T2 * V : (h + 1) * T2 * V],
                motion[h * CH : (h + 1) * CH, :],
            )

        # ---------------- setup compute (overlaps image stream) ------------
        ident = const.tile([128, 128], f32, name="ident", tag="ident")
        masks.make_identity(nc, ident[:])

        # block-diagonal att weights, zeroed once (fp16)
        lhsT_bd = const.tile([128, NG * 128], fp16, name="lhsT_bd", tag="lhsT_bd")
        nc.gpsimd.memset(lhsT_bd[:].bitcast(f32), 0.0)

        # w_iv -> lhsT_iv blocks [c'(128), d(128)] at cols (kh*2+h2)*128
        lhsT_iv = const.tile([128, 512], f32r, name="lhsT_iv", tag="lhsT_iv")
        for kh in range(2):
            for h2 in range(2):
                tp = ps_pool.tile([128, 1024], f32, name="tp", tag="ps")
                nc.tensor.transpose(
                    tp[:, 0:128],
                    w_iv_sb[:, h2 * 256 + kh * 128 : h2 * 256 + (kh + 1) * 128],
                    ident[:],
                )
                nc.scalar.copy(
                    lhsT_iv[:, (kh * 2 + h2) * 128 : (kh * 2 + h2 + 1) * 128],
                    tp[:, 0:128],
                )

        # w_iq / w_mq -> lhsT [c'(128), r(32)] at cols kh*32, pre-scaled
        lhsT_iq = const.tile([128, 64], f32r, name="lhsT_iq", tag="lhsT_iq")
        lhsT_mq = const.tile([128, 64], f32r, name="lhsT_mq", tag="lhsT_mq")
        for kh in range(2):
            tp = ps_pool.tile([128, 1024], f32, name="tp", tag="ps")
            nc.tensor.transpose(
                tp[:, 0:32], w_iq_sb[:, kh * 128 : (kh + 1) * 128], ident[0:32, 0:32]
            )
            nc.scalar.mul(lhsT_iq[:, kh * 32 : (kh + 1) * 32], tp[:, 0:32], 1.0 / HW)
            tp2 = ps_pool.tile([128, 1024], f32, name="tp2", tag="ps")
            nc.tensor.transpose(
                tp2[:, 0:32], w_mq_sb[:, kh * 128 : (kh + 1) * 128], ident[0:32, 0:32]
            )
            nc.scalar.mul(
                lhsT_mq[:, kh * 32 : (kh + 1) * 32],
                tp2[:, 0:32],
                1.0 / ((T2 // T1) * V),
            )

        # w_att [C, REL] -> lhsT_att [r(33), c(256)] fp16; row 32 = ones
        lhsT_att = const.tile([33, 256], fp16, name="lhsT_att", tag="lhsT_att")
        with nc.allow_low_precision(reason="fp16 attention weights"):
            for h in range(2):
                tp = ps_pool.tile([128, 1024], f32, name="tp", tag="ps")
                nc.tensor.transpose(
                    tp[0:32, 0:128], w_att_sb[:, h * 32 : (h + 1) * 32], ident[:]
                )
                nc.scalar.copy(
                    lhsT_att[0:32, h * 128 : (h + 1) * 128], tp[0:32, 0:128]
                )
            nc.vector.memset(lhsT_att[32:33, :], 1.0)

        # s2 [33, 256] fp16: rows 0-31 tanh scores, row 32 = 16*I(i==j)
        s2 = const.tile([33, 256], fp16, name="s2", tag="s2")
        with nc.allow_low_precision(reason="fp16 scores"):
            nc.vector.memset(s2[32:33, :], 0.0)
            for j in range(T1):
                nc.vector.memset(s2[32:33, j * 17 : j * 17 + 1], float(T1))

        # b_att pre-scaled by 1/T1
        nc.scalar.mul(b_att_sb[:], b_att_sb[:], 1.0 / T1)

        # ---------------- motion pooling + K (early, off critical path) ----
        mot_pool = const.tile([128, 32], f32r, name="mot_pool", tag="mot_pool")
        for h in range(2):
            with nc.allow_low_precision(reason="f32r output, fp32 accumulate"):
                nc.vector.reduce_sum(
                    mot_pool[:, h * T1 : (h + 1) * T1],
                    mot[:, h * T2 * V : (h + 1) * T2 * V].rearrange(
                        "p (t q) -> p t q", q=(T2 // T1) * V
                    ),
                    axis=mybir.AxisListType.X,
                )
        k_ps = ps_pool.tile([128, 1024], f32, name="k_ps", tag="ps")
        for kh in range(2):
            nc.tensor.matmul(
                k_ps[0:REL, 0:T1],
                lhsT_mq[:, kh * 32 : (kh + 1) * 32],
                mot_pool[:, kh * T1 : (kh + 1) * T1],
                start=(kh == 0),
                stop=(kh == 1),
            )
        k_sb = const.tile([REL, T1], f32, name="k_sb", tag="k_sb")
        nc.scalar.activation(
            k_sb[:], k_ps[0:REL, 0:T1], AF.Identity, bias=b_mq_sb[:, 0:1]
        )

        # ---------------- phase 1: Vv + img_sum over resident image --------
        V_fold = const.tile([128, NG * HW], fp16, name="V_fold", tag="V_fold")
        img_sum = const.tile([128, 32], f32r, name="img_sum", tag="img_sum")

        for t in range(T1):
            # spatial sums for Q (DVE)
            for h in range(2):
                with nc.allow_low_precision(reason="f32r output, fp32 accumulate"):
                    nc.vector.reduce_sum(
                        img_sum[:, h * T1 + t : h * T1 + t + 1],
                        img_sb[:, (h * T1 + t) * HW : (h * T1 + t + 1) * HW].bitcast(
                            f32
                        ),
                        axis=mybir.AxisListType.X,
                    )
            for h2 in range(2):
                ps = ps_pool.tile([128, 1024], f32, name="vv", tag="ps")
                for kh in range(2):
                    for cs, pcol in ((0, 0), (392, 512)):
                        nc.tensor.matmul(
                            ps[:, pcol : pcol + 392],
                            lhsT_iv[:, (kh * 2 + h2) * 128 : (kh * 2 + h2 + 1) * 128],
                            img_sb[
                                :,
                                (kh * T1 + t) * HW + cs : (kh * T1 + t) * HW
                                + cs
                                + 392,
                            ],
                            start=(kh == 0),
                            stop=(kh == 1),
                        )
                stg = stg_pool.tile([128, HW], fp16, name="stg", tag="stg")
                ps_view = ps[:].rearrange("p (c x) -> p c x", x=512)[:, :, 0:392]
                stg_view = stg[:].rearrange("p (c x) -> p c x", x=392)
                with nc.allow_low_precision(reason="fp16 value tensor"):
                    nc.scalar.activation(
                        stg_view, ps_view, AF.Identity, bias=b_iv_sb[:, h2 : h2 + 1]
                    )
                # fold: V_fold[16*(4*h2+cl)+t, g*HW+v] = stg[32*cl+g, v]
                nc.scalar.dma_start(
                    V_fold[:].rearrange("(cl r) q -> cl r q", r=T1)[
                        4 * h2 : 4 * h2 + 4, t
                    ],
                    stg[:],
                )

        # ---------------- phase 2: attention ------------------------------
        q_ps = ps_pool.tile([128, 1024], f32, name="q_ps", tag="ps")
        for kh in range(2):
            nc.tensor.matmul(
                q_ps[0:REL, 0:T1],
                lhsT_iq[:, kh * 32 : (kh + 1) * 32],
                img_sum[:, kh * T1 : (kh + 1) * T1],
                start=(kh == 0),
                stop=(kh == 1),
            )
        q_sb = const.tile([REL, T1], f32, name="q_sb", tag="q_sb")
        nc.scalar.activation(
            q_sb[:], q_ps[0:REL, 0:T1], AF.Identity, bias=b_iq_sb[:, 0:1]
        )

        # diff[r, j*16+i] = K[r,i] - Q[r,j]; s2[0:32] = tanh(diff) (fp16)
        diff = const.tile([REL, 256], f32, name="diff", tag="diff")
        nc.vector.tensor_sub(
            diff[:].rearrange("p (j i) -> p j i", i=T1),
            k_sb[:].unsqueeze(1).broadcast_to((REL, T1, T1)),
            q_sb[:].unsqueeze(2).broadcast_to((REL, T1, T1)),
        )
        with nc.allow_low_precision(reason="fp16 scores"):
            nc.scalar.activation(s2[0:32, :], diff[:], AF.Tanh)

        # att_sb_h[c_loc, j*16+i] = att[c, i, j] (identity + bias fused)
        att_sbs = []
        for h in range(2):
            a_ps = ps_pool.tile([128, 1024], f32, name="a_ps", tag="ps")
            nc.tensor.matmul(
                a_ps[:, 0:256],
                lhsT_att[:, h * 128 : (h + 1) * 128],
                s2[:],
            )
            att_sb = const.tile([128, 256], fp16, name="att_sb", tag=f"att_sb{h}")
            with nc.allow_low_precision(reason="fp16 attention matrix"):
                nc.scalar.activation(
                    att_sb[:],
                    a_ps[:, 0:256],
                    AF.Identity,
                    scale=1.0 / T1,
                    bias=b_att_sb[:, h : h + 1],
                )
            att_sbs.append(att_sb)

        # scatter into block-diagonal lhsT:
        # lhsT_bd[16*cl+j, g*128+16*cl+i] = att_sb[32*cl_loc+g, j*16+i]
        # dst col offset 16*cl is partition-dependent (block diagonal), so the
        # scatter cannot coalesce across cl: per-(cl, j) DMAs over 3 queues.
        engs = [nc.sync, nc.scalar, nc.gpsimd]
        k = 0
        for h in range(2):
            for j in range(T1):
                for cl_loc in range(4):
                    cl = 4 * h + cl_loc
                    s = att_sbs[h][
                        32 * cl_loc : 32 * cl_loc + 32, j * T1 : (j + 1) * T1
                    ]
                    d = lhsT_bd[16 * cl + j : 16 * cl + j + 1, :].rearrange(
                        "p (g c) -> p g c", c=128
                    )[:, :, 16 * cl : 16 * cl + 16]
                    engs[k % 3].dma_start(d, s)
                    k += 1

        # ---------------- phase 3: y = att_bd.T @ V_fold -------------------
        for g in range(NG):
            yp = ps_pool.tile([128, 1024], f32, name="yp", tag="ps")
            for cs, pcol in ((0, 0), (392, 512)):
                nc.tensor.matmul(
                    yp[:, pcol : pcol + 392],
                    lhsT_bd[:, g * 128 : (g + 1) * 128],
                    V_fold[:, g * HW + cs : g * HW + cs + 392],
                )
            y_sb = y_pool.tile([128, HW], f32, name="y_sb", tag="y_sb")
            if g % 2 == 0:
                nc.scalar.copy(
                    y_sb[:].rearrange("p (c x) -> p c x", x=392),
                    yp[:].rearrange("p (c x) -> p c x", x=512)[:, :, 0:392],
                )
            else:
                nc.vector.tensor_copy(
                    y_sb[:].rearrange("p (c x) -> p c x", x=392),
                    yp[:].rearrange("p (c x) -> p c x", x=512)[:, :, 0:392],
                )
            out_eng = nc.sync if g % 2 == 0 else nc.scalar
            out_eng.dma_start(
                y[:].rearrange("(cl g) t v -> cl g t v", g=NG)[:, g],
                y_sb[:],
            )

    nc.compile()
    return nc


def _get_nc():
    if "nc" not in _CACHE:
        _CACHE["nc"] = _build()
    return _CACHE["nc"]


def kernel(**inputs) -> np.ndarray:
    from concourse.bass_utils import run_bass_kernel_spmd

    nc = _get_nc()

    image = np.ascontiguousarray(np.asarray(inputs["image"], dtype=np.float32))
    motion = np.ascontiguousarray(np.asarray(inputs["motion"], dtype=np.float32))
    shared = {
        k: np.ascontiguousarray(np.asarray(inputs[k], dtype=np.float32))
        for k in ("w_iv", "b_iv", "w_iq", "b_iq", "w_mq", "b_mq", "w_att", "b_att")
    }
    in_maps = []
    for n in range(NCORES):
        m = {
            "image": image[n].reshape(C, T1, HW),
            "motion": motion[n].reshape(C, T2 * V),
        }
        m.update(shared)
        in_maps.append(m)

    res = run_bass_kernel_spmd(nc, in_maps, core_ids=list(range(NCORES)))
    out = np.stack([res.results[n]["y"] for n in range(NCORES)], axis=0)
    return out.reshape(N, C, T1, H, W)
